# revision 1
# baseline (speedup 1.0000x reference)
"""Trainium2 Bass kernel for CrossInnerProductWithBuyer.

Computes, per batch b (B=16384, E=128):
  out[b] = concat( windows[b] @ c[b],      # [10]
                   -(neg[b] @ c[b]),       # [64]
                   buy[b] @ c[b] )         # [1]
with c = center_vec.  Output [B, 75, 1] fp32.

Sharding: pure data-parallel over batch across 8 NeuronCores (2048
batches per core).  The host pre-transposes each core's shard so the
contraction axis e sits on the SBUF partition axis:

  at [E=128, BS*75]   columns ordered (b outer, r inner), r spanning
                      win(10) | neg(64) | buy(1)  == output order
  ct [E=128, BS]      center vectors, transposed

Per 128-batch tile the kernel then does:
  - DVE: three tensor_muls (win, neg, buy column groups) against a
    broadcast of ct / -ct  -> prod[e, (b, r)].  (The neg group uses -ct
    so the sign is folded into the product.)
  - PE:  ones[128,1]-stationary matmuls over N=512 column chunks:
    out[0, n] = sum_e prod[e, n] -- the e-reduction as a partition
    contraction.  Independent matmuls, no PSUM accumulation chains.
  - ACT: copies each PSUM strip [1, 512] to SBUF.
  - DMA: strips go out contiguously (column order == output row-major).

This keeps the DVE at exactly one pass over the data (its fp32
tensor_tensor floor), the reduction rides the otherwise-idle Tensor
engine, and GPSIMD stays idle (concurrent GPSIMD elementwise slows DVE
two-port ops ~3-5x, measured).
"""

import sys

if "/opt/trn_rl_repo" not in sys.path:
    sys.path.insert(0, "/opt/trn_rl_repo")

from contextlib import ExitStack

import numpy as np

import concourse.bass as bass
import concourse.mybir as mybir
import concourse.tile as tile
from concourse import bacc, bass_utils

B, W, N, E = 16384, 10, 64, 128
NCORES = 8
BS = B // NCORES            # 2048 batches per core
PT = 128                    # batches per tile
R = W + N + 1               # 75 output columns per batch
F = R * E                   # 9600 prod columns per tile
CHUNK = 512                 # matmul N (one PSUM bank of fp32)
STRIP = 2048                # PSUM strip: 4 chunks copied/stored together

FP32 = mybir.dt.float32


def _build(bs: int = BS) -> bass.Bass:
    nt = bs // PT
    nc = bacc.Bacc("TRN2", target_bir_lowering=False, debug=False,
                   num_devices=NCORES)
    at = nc.dram_tensor("at", [E, bs * R], FP32, kind="ExternalInput").ap()
    ct = nc.dram_tensor("ct", [E, bs], FP32, kind="ExternalInput").ap()
    out = nc.dram_tensor("out", [1, bs * R], FP32, kind="ExternalOutput").ap()

    with tile.TileContext(nc) as tc, ExitStack() as ctx:
        apool = ctx.enter_context(tc.tile_pool(name="a", bufs=4))
        cpool = ctx.enter_context(tc.tile_pool(name="c", bufs=4))
        ncpool = ctx.enter_context(tc.tile_pool(name="negc", bufs=4))
        spool = ctx.enter_context(tc.tile_pool(name="strip", bufs=3))
        pspool = ctx.enter_context(tc.tile_pool(name="ps", bufs=2,
                                                space="PSUM"))
        onepool = ctx.enter_context(tc.tile_pool(name="ones", bufs=1))

        ones = onepool.tile([E, 1], FP32)
        nc.vector.memset(ones[:], 1.0)

        for t in range(nt):
            col0 = t * F
            a = apool.tile([E, F], FP32)
            nc.sync.dma_start(a[:], at[:, col0:col0 + F])
            c = cpool.tile([E, PT], FP32)
            nc.sync.dma_start(c[:], ct[:, t * PT:(t + 1) * PT])
            negc = ncpool.tile([E, PT], FP32)
            nc.vector.tensor_scalar_mul(negc[:], c[:], -1.0)

            # a viewed as [e, b, r]; multiply r-groups by (+-)c[e, b],
            # in place (the product overwrites a, saving an SBUF buffer).
            av = a[:].rearrange("e (b r) -> e b r", r=R)
            p = a
            nc.vector.tensor_mul(
                av[:, :, 0:W], av[:, :, 0:W],
                c[:].unsqueeze(2).broadcast_to([E, PT, W]))
            nc.vector.tensor_mul(
                av[:, :, W:W + N], av[:, :, W:W + N],
                negc[:].unsqueeze(2).broadcast_to([E, PT, N]))
            nc.vector.tensor_mul(
                av[:, :, W + N:R], av[:, :, W + N:R],
                c[:].unsqueeze(2).broadcast_to([E, PT, 1]))

            # e-reduction on the Tensor engine: ones.T @ prod chunk.
            # 4 matmuls (N=512, one PSUM bank each) fill a 4-bank strip;
            # one ACT copy + one DMA per strip keeps the sem-chain short.
            for g0 in range(0, F, STRIP):
                gn = min(STRIP, F - g0)
                ps = pspool.tile([1, STRIP], FP32)
                for k0 in range(0, gn, CHUNK):
                    n = min(CHUNK, gn - k0)
                    nc.tensor.matmul(ps[:, k0:k0 + n], ones[:],
                                     p[:, g0 + k0:g0 + k0 + n],
                                     start=True, stop=True)
                s = spool.tile([1, STRIP], FP32)
                nc.scalar.copy(s[:, 0:gn], ps[:, 0:gn])
                nc.scalar.dma_start(out[:, col0 + g0:col0 + g0 + gn],
                                    s[:, 0:gn])
    nc.compile()
    return nc


_NC_CACHE: dict = {}


def _get_nc(bs: int = BS) -> bass.Bass:
    if bs not in _NC_CACHE:
        _NC_CACHE[bs] = _build(bs)
    return _NC_CACHE[bs]


def _prep_core(center, windows, negs, buy):
    """Transpose one core's shard to the kernel's (e-major) layout."""
    bs = center.shape[0]
    a = np.concatenate([
        windows.reshape(bs, W, E),
        negs.reshape(bs, N, E),
        buy.reshape(bs, 1, E),
    ], axis=1)                                   # [bs, 75, E]
    at = np.ascontiguousarray(a.transpose(2, 0, 1).reshape(E, bs * R),
                              dtype=np.float32)
    ct = np.ascontiguousarray(center.reshape(bs, E).T, dtype=np.float32)
    return at, ct


def _shard_inputs(center_vec, windows_vecs, neg_vecs, buy_vec):
    center_vec = np.asarray(center_vec, dtype=np.float32)
    windows_vecs = np.asarray(windows_vecs, dtype=np.float32)
    neg_vecs = np.asarray(neg_vecs, dtype=np.float32)
    buy_vec = np.asarray(buy_vec, dtype=np.float32)
    in_maps = []
    for i in range(NCORES):
        sl = slice(i * BS, (i + 1) * BS)
        at, ct = _prep_core(center_vec[sl], windows_vecs[sl],
                            neg_vecs[sl], buy_vec[sl])
        in_maps.append({"at": at, "ct": ct})
    return in_maps


def run(center_vec, windows_vecs, neg_vecs, buy_vec, trace: bool = False):
    """Run on 8 NeuronCores; returns (full_output, BassKernelResults)."""
    nc = _get_nc()
    in_maps = _shard_inputs(center_vec, windows_vecs, neg_vecs, buy_vec)
    res = bass_utils.run_bass_kernel_spmd(
        nc, in_maps, list(range(NCORES)), trace=trace)
    full = np.concatenate(
        [res.results[i]["out"].reshape(BS, R) for i in range(NCORES)], axis=0)
    return full.reshape(B, R, 1), res


def kernel(center_vec, windows_vecs, neg_vecs, buy_vec):
    out, _ = run(center_vec, windows_vecs, neg_vecs, buy_vec)
    return out



# revision 2
# speedup vs baseline: 2.1692x; 2.1692x over previous
"""Trainium2 Bass kernel for CrossInnerProductWithBuyer.

Computes, per batch b (B=16384, E=128):
  out[b] = concat( windows[b] @ c[b],      # [10]
                   -(neg[b] @ c[b]),       # [64]
                   buy[b] @ c[b] )         # [1]
with c = center_vec.  Output [B, 75, 1] fp32.

Sharding: pure data-parallel over batch across 8 NeuronCores (2048
batches per core).  Memory-bound problem (~608 MB of input), so inputs
are cast to fp16 on the host (tolerance gate is 2e-2; fp16 dot error is
~1e-4 relative): halves DMA bytes, and fp16 runs the PE at 1 cycle/col
(vs 4 for fp32) and the DVE in 2x mode.

Host pre-negates the neg block and pre-transposes each core's shard so
the contraction axis e sits on the SBUF partition axis, with tile
columns ordered (r outer, b inner):

  at [E=128, BS*75]   col (t, r, b) = a[t*128+b, r, :]  where a is
                      concat(win, -neg, buy) along r
  ct [E=128, BS]      center vectors, transposed

Per 128-batch tile:
  - DVE: ONE in-place tensor_mul of the [E, 75, 128] tile against
    ct[:, tile] broadcast over r.  (r-outer ordering keeps the
    broadcast's innermost axis stride-1, required for DVE 2x mode.)
  - PE:  20 matmuls of 480 columns each, chunk j using a "shifted ones"
    stationary [128, 20] (ones in column j only), all accumulating into
    one PSUM region [20, 480] -> chunk j's column sums land on PSUM
    partition j.
  - ACT: one [20, 480] PSUM->SBUF copy per tile (multi-partition, vs
    the pathological [1, N] single-partition copy).
  - DMA: one [20, 480] store per tile; host untangles (r, b) -> (b, r).
"""

import sys

if "/opt/trn_rl_repo" not in sys.path:
    sys.path.insert(0, "/opt/trn_rl_repo")

from contextlib import ExitStack

import numpy as np

import concourse.bass as bass
import concourse.mybir as mybir
import concourse.tile as tile
from concourse import bacc, bass_utils

B, W, N, E = 16384, 10, 64, 128
NCORES = 8
BS = B // NCORES            # 2048 batches per core
PT = 128                    # batches per tile
NT = BS // PT               # 16 tiles per core
R = W + N + 1               # 75 output rows per batch
F = R * PT                  # 9600 product columns per tile
CHUNK = 480                 # matmul N; 20 * 480 == F, 480*4B < 2KB bank
NCH = F // CHUNK            # 20 chunks -> PSUM partitions 0..19

FP32 = mybir.dt.float32
FP16 = mybir.dt.float16


def _build(bs: int = BS) -> bass.Bass:
    nt = bs // PT
    nc = bacc.Bacc("TRN2", target_bir_lowering=False, debug=False,
                   num_devices=NCORES)
    at = nc.dram_tensor("at", [E, bs * R], FP16, kind="ExternalInput").ap()
    ct = nc.dram_tensor("ct", [E, bs], FP16, kind="ExternalInput").ap()
    out = nc.dram_tensor("out", [nt * NCH, CHUNK], FP32,
                         kind="ExternalOutput").ap()

    with tile.TileContext(nc) as tc, ExitStack() as ctx:
        apool = ctx.enter_context(tc.tile_pool(name="a", bufs=5))
        cpool = ctx.enter_context(tc.tile_pool(name="c", bufs=1))
        idpool = ctx.enter_context(tc.tile_pool(name="id", bufs=1))
        spool = ctx.enter_context(tc.tile_pool(name="stage", bufs=4))
        pspool = ctx.enter_context(tc.tile_pool(name="ps", bufs=4,
                                                space="PSUM"))

        cfull = cpool.tile([E, bs], FP16)
        nc.sync.dma_start(cfull[:], ct[:])

        # Stationary bank: idv[:, j, :] is [128, 20] with ones in column
        # j only -> matmul routes chunk j's column sums to PSUM row j.
        idt = idpool.tile([E, NCH * NCH], FP16)
        nc.vector.memset(idt[:], 0.0)
        idv = idt[:].rearrange("e (j m) -> e j m", m=NCH)
        for j in range(NCH):
            nc.vector.memset(idv[:, j, j:j + 1], 1.0)

        for t in range(nt):
            a = apool.tile([E, F], FP16)
            nc.sync.dma_start(a[:], at[:, t * F:(t + 1) * F])

            av = a[:].rearrange("e (r b) -> e r b", b=PT)
            cb = cfull[:, t * PT:(t + 1) * PT].unsqueeze(1) \
                .broadcast_to([E, R, PT])
            nc.vector.tensor_mul(av, av, cb)

            ps = pspool.tile([NCH, CHUNK], FP32)
            for j in range(NCH):
                nc.tensor.matmul(ps[:], idv[:, j, :],
                                 a[:, j * CHUNK:(j + 1) * CHUNK],
                                 start=(j == 0), stop=(j == NCH - 1))

            st = spool.tile([NCH, CHUNK], FP32)
            nc.scalar.copy(st[:], ps[:])
            nc.scalar.dma_start(out[t * NCH:(t + 1) * NCH, :], st[:])
    nc.compile()
    return nc


_NC_CACHE: dict = {}


def _get_nc(bs: int = BS) -> bass.Bass:
    if bs not in _NC_CACHE:
        _NC_CACHE[bs] = _build(bs)
    return _NC_CACHE[bs]


def _prep_core(center, windows, negs, buy):
    """Cast one core's shard to fp16 in the kernel's (e-major, r-outer
    b-inner) layout, with the neg block pre-negated."""
    bs = center.shape[0]
    a = np.concatenate([
        windows.reshape(bs, W, E).astype(np.float16),
        -(negs.reshape(bs, N, E).astype(np.float16)),
        buy.reshape(bs, 1, E).astype(np.float16),
    ], axis=1)                                   # [bs, 75, E] fp16
    at = np.ascontiguousarray(
        a.reshape(bs // PT, PT, R, E).transpose(3, 0, 2, 1).reshape(
            E, bs * R))
    ct = np.ascontiguousarray(center.reshape(bs, E).astype(np.float16).T)
    return at, ct


def _shard_inputs(center_vec, windows_vecs, neg_vecs, buy_vec):
    center_vec = np.asarray(center_vec, dtype=np.float32)
    windows_vecs = np.asarray(windows_vecs, dtype=np.float32)
    neg_vecs = np.asarray(neg_vecs, dtype=np.float32)
    buy_vec = np.asarray(buy_vec, dtype=np.float32)
    in_maps = []
    for i in range(NCORES):
        sl = slice(i * BS, (i + 1) * BS)
        at, ct = _prep_core(center_vec[sl], windows_vecs[sl],
                            neg_vecs[sl], buy_vec[sl])
        in_maps.append({"at": at, "ct": ct})
    return in_maps


def run(center_vec, windows_vecs, neg_vecs, buy_vec, trace: bool = False):
    """Run on 8 NeuronCores; returns (full_output, BassKernelResults)."""
    nc = _get_nc()
    in_maps = _shard_inputs(center_vec, windows_vecs, neg_vecs, buy_vec)
    res = bass_utils.run_bass_kernel_spmd(
        nc, in_maps, list(range(NCORES)), trace=trace)
    parts = []
    for i in range(NCORES):
        o = res.results[i]["out"].reshape(NT, R, PT)
        parts.append(np.ascontiguousarray(o.transpose(0, 2, 1)).reshape(
            BS, R))
    full = np.concatenate(parts, axis=0)
    return full.reshape(B, R, 1), res


def kernel(center_vec, windows_vecs, neg_vecs, buy_vec):
    out, _ = run(center_vec, windows_vecs, neg_vecs, buy_vec)
    return out
